# revision 1
# baseline (speedup 1.0000x reference)
"""GIN message-passing GNN on 8 Trainium2 NeuronCores (Bass/Tile).

Strategy (self-contained; shapes hardcoded for the 100k-node / 1.6M-edge /
128-dim / 10-layer / 64-graph problem):

- Nodes are partitioned into 8 contiguous ranges of 12500; each core owns the
  edges whose *destination* falls in its range.
- Each core keeps a full replica of the node features in its HBM. Per layer it
  gathers x[src] for its ~200k edges with one indirect DMA per 128-dst tile
  (edges pre-sorted by dst tile host-side, padded to a multiple of 128 with
  src=0 / dstoff=-1; pad length is the max over cores so the program is SPMD).
- The segment-sum (scatter-add) becomes a matmul: for each 128-edge chunk,
  PSUM[feat, dst] += contract_edges(gathered[edge, feat], onehot[edge, dst]),
  with the one-hot built on the vector engine by a broadcast is_equal against
  a resident iota row. Padding lanes have dstoff=-1 -> all-zero column.
- The GIN MLP runs in the transposed [feat, node] orientation so both matmuls
  chain without transposes; the per-core x^T slab stays resident in SBUF for
  the residual adds. Only the final per-tile result is transposed (tensor
  engine) for the HBM store.
- After each layer an AllGather over the 8 cores rebuilds the full replica.
- Mean-pool: during the last layer, each [node, feat] output tile is reduced
  into a PSUM[feat, graph] accumulator via a matmul against a graph-selection
  one-hot built from a per-core batch-id input; scale by 1/count, AllReduce,
  then the tiny classifier MLP on every core.
"""
import os
import sys

sys.path.insert(0, "/opt/trn_rl_repo")

import numpy as np

N_NODES = 100000
N_EDGES = 1600000
F = 128
NUM_LAYERS = int(os.environ.get("GNN_LAYERS", "10"))
NUM_GRAPHS = 64
NUM_CLASSES = 2
N_CORES = 8
NPC = N_NODES // N_CORES          # 12500 nodes per core
NT = (NPC + 127) // 128           # 98 dst tiles per core
LAST_W = NPC - (NT - 1) * 128     # 84 nodes in the last tile

_CACHE = {}


def _preprocess(edge_index, batch):
    """Host-side: per-core gather indices / dst offsets in the SBUF layout."""
    src = np.ascontiguousarray(edge_index[0]).astype(np.int64)
    dst = np.ascontiguousarray(edge_index[1]).astype(np.int64)

    order = np.argsort(dst, kind="stable")
    s_src = src[order].astype(np.int32)
    s_dst = dst[order]

    # node-id boundaries of every (core, tile)
    tile_starts = (np.arange(N_CORES)[:, None] * NPC
                   + np.minimum(np.arange(NT + 1)[None, :] * 128, NPC))
    bounds = np.searchsorted(s_dst, tile_starts.ravel()).reshape(N_CORES, NT + 1)
    counts = np.diff(bounds, axis=1)                      # [N_CORES, NT]

    padded = np.maximum(counts.max(axis=0), 1)
    padded = ((padded + 127) // 128) * 128                # per-tile padded len
    nch = (padded // 128).astype(np.int64)                # 128-chunks per tile
    colbase = np.concatenate([[0], np.cumsum(nch)])
    C_total = int(colbase[-1])

    gidx = np.zeros((N_CORES, 128, C_total), dtype=np.int32)
    gdst = np.full((N_CORES, 128, C_total), -1.0, dtype=np.float32)
    for c in range(N_CORES):
        lo, hi = bounds[c, 0], bounds[c, NT]
        e_src = s_src[lo:hi]
        local = s_dst[lo:hi] - c * NPC
        t_e = local // 128
        grp_start = np.repeat(bounds[c, :-1] - lo, counts[c])
        r = np.arange(hi - lo) - grp_start                # rank within tile
        p = r % 128
        col = colbase[t_e] + r // 128
        gidx[c, p, col] = e_src
        gdst[c, p, col] = (local % 128).astype(np.float32)

    # per-core local batch ids, [128, NT], padding rows = -1
    blocal = np.full((N_CORES, 128, NT), -1.0, dtype=np.float32)
    b = np.asarray(batch).astype(np.int64)
    for c in range(N_CORES):
        ids = b[c * NPC:(c + 1) * NPC].astype(np.float32)
        ids = np.concatenate([ids, np.full(NT * 128 - NPC, -1.0, np.float32)])
        blocal[c] = ids.reshape(NT, 128).T

    cnt = np.bincount(b, minlength=NUM_GRAPHS).astype(np.float64)
    inv = (1.0 / np.maximum(cnt, 1.0)).astype(np.float32)
    return gidx, gdst, nch, colbase, blocal, inv


def _build(nch, colbase):
    from concourse import bacc, bass, mybir
    import concourse.tile as tile

    f32 = mybir.dt.float32
    C_total = int(colbase[-1])

    nc = bacc.Bacc("TRN2", target_bir_lowering=False, debug=False,
                   num_devices=N_CORES)

    # ---- I/O ----
    x_in = nc.dram_tensor("x", [N_NODES, F], f32, kind="ExternalInput")
    xT_in = nc.dram_tensor("xT_own", [F, NPC], f32, kind="ExternalInput")
    gidx_in = nc.dram_tensor("gidx", [128, C_total], mybir.dt.int32,
                             kind="ExternalInput")
    gdst_in = nc.dram_tensor("gdst", [128, C_total], f32, kind="ExternalInput")
    bl_in = nc.dram_tensor("blocal", [128, NT], f32, kind="ExternalInput")
    iota_in = nc.dram_tensor("iota", [128, 128], f32, kind="ExternalInput")
    ident_in = nc.dram_tensor("ident", [128, 128], f32, kind="ExternalInput")
    w1_in = nc.dram_tensor("w1cat", [F, NUM_LAYERS * F], f32, kind="ExternalInput")
    w2_in = nc.dram_tensor("w2cat", [F, NUM_LAYERS * F], f32, kind="ExternalInput")
    b1_in = nc.dram_tensor("b1t", [F, NUM_LAYERS], f32, kind="ExternalInput")
    b2_in = nc.dram_tensor("b2t", [F, NUM_LAYERS], f32, kind="ExternalInput")
    eps_in = nc.dram_tensor("epsb", [F, NUM_LAYERS], f32, kind="ExternalInput")
    wc1_in = nc.dram_tensor("wc1", [F, F], f32, kind="ExternalInput")
    bc1_in = nc.dram_tensor("bc1c", [F, 1], f32, kind="ExternalInput")
    wc2_in = nc.dram_tensor("wc2", [F, NUM_CLASSES], f32, kind="ExternalInput")
    bc2_in = nc.dram_tensor("bc2c", [NUM_CLASSES, 1], f32, kind="ExternalInput")
    inv_in = nc.dram_tensor("invc", [128, NUM_GRAPHS], f32, kind="ExternalInput")
    out_t = nc.dram_tensor("logits_t", [NUM_CLASSES, NUM_GRAPHS], f32,
                           kind="ExternalOutput")

    # ---- internal DRAM ----
    x_rep = nc.dram_tensor("x_rep", [N_NODES, F], f32, kind="Internal")
    newx = nc.dram_tensor("newx", [NPC, F], f32, kind="Internal")
    pr_in = nc.dram_tensor("pr_in", [128, NUM_GRAPHS], f32, kind="Internal")
    pr_out = nc.dram_tensor("pr_out", [128, NUM_GRAPHS], f32, kind="Internal")

    rg = [list(range(N_CORES))]

    with tile.TileContext(nc) as tc:
        from contextlib import ExitStack
        ctx = ExitStack()
        const = ctx.enter_context(tc.tile_pool(name="const", bufs=1))
        gpool = ctx.enter_context(tc.tile_pool(name="gather", bufs=3))
        opool = ctx.enter_context(tc.tile_pool(name="onehot", bufs=3))
        wpool = ctx.enter_context(tc.tile_pool(name="work", bufs=3))
        psum = ctx.enter_context(tc.tile_pool(name="psum", bufs=2, space="PSUM"))

        xT_res = const.tile([F, NPC], f32)
        gidx_t = const.tile([128, C_total], mybir.dt.int32)
        gdst_t = const.tile([128, C_total], f32)
        bl_t = const.tile([128, NT], f32)
        iota_t = const.tile([128, 128], f32)
        ident_t = const.tile([128, 128], f32)
        w1_t = const.tile([F, NUM_LAYERS * F], f32)
        w2_t = const.tile([F, NUM_LAYERS * F], f32)
        b1_t = const.tile([F, NUM_LAYERS], f32)
        b2_t = const.tile([F, NUM_LAYERS], f32)
        eps_t = const.tile([F, NUM_LAYERS], f32)
        wc1_t = const.tile([F, F], f32)
        bc1_t = const.tile([F, 1], f32)
        wc2_t = const.tile([F, NUM_CLASSES], f32)
        bc2_t = const.tile([NUM_CLASSES, 1], f32)
        inv_t = const.tile([128, NUM_GRAPHS], f32)

        for tle, src_t in [(xT_res, xT_in), (gdst_t, gdst_in), (bl_t, bl_in),
                           (iota_t, iota_in), (ident_t, ident_in),
                           (w1_t, w1_in), (w2_t, w2_in), (b1_t, b1_in),
                           (b2_t, b2_in), (eps_t, eps_in), (wc1_t, wc1_in),
                           (bc1_t, bc1_in), (wc2_t, wc2_in), (bc2_t, bc2_in),
                           (inv_t, inv_in)]:
            nc.sync.dma_start(tle[:], src_t[:])
        nc.sync.dma_start(gidx_t[:], gidx_in[:])

        pool_ps = psum.tile([F, NUM_GRAPHS], f32, tag="pool", bufs=1)

        for layer in range(NUM_LAYERS):
            src_dram = x_in if layer == 0 else x_rep
            last = layer == NUM_LAYERS - 1
            for t in range(NT):
                tw = 128 if t < NT - 1 else LAST_W
                n = int(nch[t])
                cb = int(colbase[t])
                ts = t * 128

                gbuf = gpool.tile([128, n, F], f32, tag="gbuf")
                for j in range(n):
                    # HW contract: one offset per partition, 128 rows/call
                    nc.gpsimd.indirect_dma_start(
                        out=gbuf[:, j, :],
                        out_offset=None,
                        in_=src_dram[:],
                        in_offset=bass.IndirectOffsetOnAxis(
                            ap=gidx_t[:, cb + j:cb + j + 1], axis=0),
                    )

                oh = opool.tile([128, n, 128], f32, tag="oh")
                nc.vector.tensor_tensor(
                    out=oh[:],
                    in0=gdst_t[:, cb:cb + n, None].to_broadcast([128, n, 128]),
                    in1=iota_t[:, None, :].to_broadcast([128, n, 128]),
                    op=mybir.AluOpType.is_equal,
                )

                aggr = psum.tile([F, 128], f32, tag="aggr", bufs=2)
                for j in range(n):
                    nc.tensor.matmul(aggr[:], gbuf[:, j, :], oh[:, j, :],
                                     start=(j == 0), stop=(j == n - 1))

                xT_sl = xT_res[:, ts:ts + tw]
                h = wpool.tile([F, 128], f32, tag="h")
                nc.vector.tensor_scalar(
                    out=h[:, :tw], in0=xT_sl, scalar1=eps_t[:, layer:layer + 1],
                    scalar2=None, op0=mybir.AluOpType.mult)
                nc.vector.tensor_tensor(
                    out=h[:, :tw], in0=h[:, :tw], in1=aggr[:, :tw],
                    op=mybir.AluOpType.add)

                p1 = psum.tile([F, 128], f32, tag="p1", bufs=1)
                nc.tensor.matmul(p1[:, :tw], w1_t[:, layer * F:(layer + 1) * F],
                                 h[:, :tw], start=True, stop=True)
                r1 = wpool.tile([F, 128], f32, tag="r1")
                nc.scalar.activation(r1[:, :tw], p1[:, :tw],
                                     mybir.ActivationFunctionType.Relu,
                                     bias=b1_t[:, layer:layer + 1])

                p2 = psum.tile([F, 128], f32, tag="p2", bufs=1)
                nc.tensor.matmul(p2[:, :tw], w2_t[:, layer * F:(layer + 1) * F],
                                 r1[:, :tw], start=True, stop=True)

                o = wpool.tile([F, 128], f32, tag="o")
                if layer > 0:
                    nc.vector.tensor_tensor(out=o[:, :tw], in0=p2[:, :tw],
                                            in1=h[:, :tw],
                                            op=mybir.AluOpType.add)
                    nc.scalar.activation(o[:, :tw], o[:, :tw],
                                         mybir.ActivationFunctionType.Relu,
                                         bias=b2_t[:, layer:layer + 1])
                else:
                    nc.scalar.activation(o[:, :tw], p2[:, :tw],
                                         mybir.ActivationFunctionType.Relu,
                                         bias=b2_t[:, layer:layer + 1])
                nc.vector.tensor_tensor(out=xT_sl, in0=o[:, :tw], in1=xT_sl,
                                        op=mybir.AluOpType.add)

                pt = psum.tile([128, F], f32, tag="pt", bufs=2)
                nc.tensor.transpose(out=pt[:tw, :], in_=xT_res[:, ts:ts + tw],
                                    identity=ident_t[:])
                st = wpool.tile([128, F], f32, tag="st")
                nc.vector.tensor_copy(st[:tw, :], pt[:tw, :])
                if not last:
                    nc.sync.dma_start(newx[ts:ts + tw, :], st[:tw, :])
                else:
                    # fold this tile into the pooling accumulator
                    sel = wpool.tile([128, NUM_GRAPHS], f32, tag="sel")
                    nc.vector.tensor_tensor(
                        out=sel[:],
                        in0=bl_t[:, t:t + 1].to_broadcast([128, NUM_GRAPHS]),
                        in1=iota_t[:, :NUM_GRAPHS],
                        op=mybir.AluOpType.is_equal,
                    )
                    nc.tensor.matmul(pool_ps[:], st[:], sel[:],
                                     start=(t == 0), stop=(t == NT - 1))

            if not last:
                nc.gpsimd.collective_compute(
                    "AllGather", mybir.AluOpType.bypass,
                    ins=[newx[:]], outs=[x_rep[:]], replica_groups=rg)

        # ---- mean pool + classifier ----
        pacc = wpool.tile([128, NUM_GRAPHS], f32, tag="pacc")
        nc.vector.tensor_tensor(out=pacc[:], in0=pool_ps[:], in1=inv_t[:],
                                op=mybir.AluOpType.mult)
        nc.sync.dma_start(pr_in[:], pacc[:])
        nc.gpsimd.collective_compute(
            "AllReduce", mybir.AluOpType.add,
            ins=[pr_in[:]], outs=[pr_out[:]], replica_groups=rg)
        pooled = wpool.tile([128, NUM_GRAPHS], f32, tag="pooled")
        nc.sync.dma_start(pooled[:], pr_out[:])

        pc1 = psum.tile([F, NUM_GRAPHS], f32, tag="aggr", bufs=2)
        nc.tensor.matmul(pc1[:], wc1_t[:], pooled[:], start=True, stop=True)
        rc1 = wpool.tile([F, NUM_GRAPHS], f32, tag="rc1")
        nc.scalar.activation(rc1[:], pc1[:], mybir.ActivationFunctionType.Relu,
                             bias=bc1_t[:])
        pc2 = psum.tile([NUM_CLASSES, NUM_GRAPHS], f32, tag="p1", bufs=1)
        nc.tensor.matmul(pc2[:], wc2_t[:], rc1[:], start=True, stop=True)
        lg = wpool.tile([NUM_CLASSES, NUM_GRAPHS], f32, tag="lg")
        nc.vector.tensor_scalar(out=lg[:], in0=pc2[:], scalar1=bc2_t[:],
                                scalar2=None, op0=mybir.AluOpType.add)
        nc.sync.dma_start(out_t[:], lg[:])
        ctx.close()

    nc.compile()
    return nc


def _get_module(nch, colbase):
    key = tuple(nch.tolist())
    if key not in _CACHE:
        _CACHE.clear()
        _CACHE[key] = _build(nch, colbase)
    return _CACHE[key]


def kernel(x, edge_index, batch, eps, W1, b1, W2, b2, Wc1, bc1, Wc2, bc2,
           _trace=False):
    from concourse.bass_utils import run_bass_kernel_spmd

    x = np.ascontiguousarray(np.asarray(x), dtype=np.float32)
    eps = np.asarray(eps, dtype=np.float32)
    W1 = np.asarray(W1, dtype=np.float32)
    b1 = np.asarray(b1, dtype=np.float32)
    W2 = np.asarray(W2, dtype=np.float32)
    b2 = np.asarray(b2, dtype=np.float32)

    gidx, gdst, nch, colbase, blocal, inv = _preprocess(
        np.asarray(edge_index), np.asarray(batch))
    nc = _get_module(nch, colbase)

    L = NUM_LAYERS
    common = {
        "x": x,
        "iota": np.ascontiguousarray(
            np.broadcast_to(np.arange(128, dtype=np.float32), (128, 128))),
        "ident": np.eye(128, dtype=np.float32),
        "w1cat": np.ascontiguousarray(np.concatenate(list(W1[:L]), axis=1)),
        "w2cat": np.ascontiguousarray(np.concatenate(list(W2[:L]), axis=1)),
        "b1t": np.ascontiguousarray(b1[:L].T),
        "b2t": np.ascontiguousarray(b2[:L].T),
        "epsb": np.ascontiguousarray(
            np.broadcast_to(1.0 + eps[:L], (F, L))),
        "wc1": np.ascontiguousarray(np.asarray(Wc1, np.float32)),
        "bc1c": np.ascontiguousarray(np.asarray(bc1, np.float32)[:, None]),
        "wc2": np.ascontiguousarray(np.asarray(Wc2, np.float32)),
        "bc2c": np.ascontiguousarray(np.asarray(bc2, np.float32)[:, None]),
        "invc": np.ascontiguousarray(np.broadcast_to(inv, (128, NUM_GRAPHS))),
    }
    in_maps = []
    for c in range(N_CORES):
        m = dict(common)
        m["xT_own"] = np.ascontiguousarray(x[c * NPC:(c + 1) * NPC].T)
        m["gidx"] = gidx[c]
        m["gdst"] = gdst[c]
        m["blocal"] = blocal[c]
        in_maps.append(m)

    res = run_bass_kernel_spmd(nc, in_maps, core_ids=list(range(N_CORES)),
                               trace=_trace)
    out = np.ascontiguousarray(res.results[0]["logits_t"].T)
    if _trace:
        kernel._last_result = res
    return out



# revision 5
# speedup vs baseline: 1.2832x; 1.2832x over previous
"""GIN message-passing GNN on 8 Trainium2 NeuronCores (Bass/Tile).

V2 strategy (self-contained; shapes hardcoded for the 100k-node / 1.6M-edge /
128-dim / 10-layer / 64-graph problem):

- Nodes partitioned into 8 contiguous ranges of 12500; each core owns the
  edges whose destination falls in its range. Node state is kept in bf16.
- Per layer, each core gathers x[src] for its ~200k edges with a few large
  GPSIMD dma_gather calls (int16 indices => sources are split into 4
  "quarters" of 25000 nodes; per destination-tile the edges are bucketed by
  source quarter, each (tile, quarter) bucket padded to a 128-edge-chunk
  boundary with dummy index 0; padding is the max over cores so the program
  is SPMD). Calls cover groups of G tiles -> ~4*ceil(98/G) gather
  instructions per layer instead of ~1600 indirect DMAs.
- The segment-sum becomes a matmul per 128-edge chunk:
  PSUM[feat, dst] += contract_edges(gathered[edge, feat], onehot[edge, dst])
  with the bf16 one-hot built on the vector engine by is_equal against a
  resident iota row (dummy slots have dst=-1 -> zero column).
- The GIN MLP runs in the transposed [feat, node] orientation in bf16 (fp32
  PSUM accumulation); the per-core x^T slab stays resident in SBUF in fp32
  for exact residual adds. Output tiles are cast to bf16, transposed on the
  tensor engine, and stored to a local newx slab.
- After each layer an AllGather with a *Shared* HBM output rebuilds the full
  replica once (each core only contributes its 3.2MB slab instead of
  materializing a private 25.6MB copy). Replicas rotate over 3 shared
  buffers to avoid cross-layer write-after-read hazards.
- Mean-pool: last layer's output tiles are reduced into PSUM[feat, graph]
  via a matmul against a graph one-hot; scale by 1/count, AllReduce, then
  the small classifier MLP on every core.
"""
import os
import sys

sys.path.insert(0, "/opt/trn_rl_repo")

import numpy as np

N_NODES = 100000
N_EDGES = 1600000
F = 128
NUM_LAYERS = int(os.environ.get("GNN_LAYERS", "10"))
NUM_GRAPHS = 64
NUM_CLASSES = 2
N_CORES = 8
NPC = N_NODES // N_CORES          # 12500 nodes per core
NT = (NPC + 127) // 128           # 98 dst tiles per core
LAST_W = NPC - (NT - 1) * 128     # 84 nodes in the last tile
NQ = 4                            # source quarters (int16 idx limit)
QR = N_NODES // NQ                # 25000 rows per quarter
GTILES = 6                        # dst tiles per gather group

_CACHE = {}


def _preprocess(edge_index, batch):
    """Host-side: per-core gather index streams / dst-offset streams.

    Returns (gidx16, gdst, layout, blocal, inv) where layout carries all the
    static column bookkeeping shared by every core.
    """
    src = np.ascontiguousarray(edge_index[0]).astype(np.int64)
    dst = np.ascontiguousarray(edge_index[1]).astype(np.int64)

    # sort edges by (core, tile, quarter) with one composite stable sort
    gt = (dst // NPC) * NT + (dst % NPC) // 128      # global tile 0..783
    key = gt * NQ + src // QR
    order = np.argsort(key, kind="stable")
    e_src = src[order]
    e_dst = dst[order]
    e_key = key[order]

    counts = np.bincount(e_key, minlength=N_CORES * NT * NQ)
    counts = counts.reshape(N_CORES, NT, NQ)
    pc = (counts.max(axis=0) + 127) // 128            # [NT, NQ] chunks
    n_t = pc.sum(axis=1)                              # [NT]

    # group layout: group g covers tiles [g*GTILES, ...); within a group the
    # gather-call (gbuf) column order is q-major: q0:[t0,t1,..] q1:[...] ...
    ngroups = (NT + GTILES - 1) // GTILES
    groups = [list(range(g * GTILES, min((g + 1) * GTILES, NT)))
              for g in range(ngroups)]

    # tile-major (one-hot / gdst) global column base
    toffs = np.concatenate([[0], np.cumsum(n_t)]).astype(np.int64)
    C_total = int(toffs[-1])

    # gbuf column bookkeeping
    gb_groupbase = []          # global gbuf col where group g starts
    gb_callbase = []           # [g][q] global gbuf col where call (g,q) starts
    gb_tilebase = []           # [g][q][t] global gbuf col of tile t's q-chunks
    pos = 0
    for g, tl in enumerate(groups):
        gb_groupbase.append(pos)
        cb = []
        tb = []
        for q in range(NQ):
            cb.append(pos)
            tbq = {}
            for t in tl:
                tbq[t] = pos
                pos += int(pc[t, q])
            tb.append(tbq)
        gb_callbase.append(cb)
        gb_tilebase.append(tb)
    assert pos == C_total

    # per-edge positions (same math for every core; data differs)
    g_of_t = np.zeros(NT, dtype=np.int64)
    for g, tl in enumerate(groups):
        for t in tl:
            g_of_t[t] = g
    tile_of_e = (e_dst % NPC) // 128
    q_of_e = e_src // QR
    core_of_e = e_dst // NPC

    # rank of each edge within its (c,t,q) bucket
    bucket_sizes = counts.reshape(-1)
    bucket_start = np.concatenate([[0], np.cumsum(bucket_sizes)])[:-1]
    flat_key = (core_of_e * NT + tile_of_e) * NQ + q_of_e
    r = np.arange(len(e_src)) - bucket_start[flat_key]

    # gbuf slot (within-call): (tile's col base - call base)*128 + r
    gb_tile_col = np.zeros((NT, NQ), dtype=np.int64)
    call_col = np.zeros((NT, NQ), dtype=np.int64)
    for g, tl in enumerate(groups):
        for q in range(NQ):
            for t in tl:
                gb_tile_col[t, q] = gb_tilebase[g][q][t]
                call_col[t, q] = gb_callbase[g][q]
    slot_in_call = (gb_tile_col[tile_of_e, q_of_e]
                    - call_col[tile_of_e, q_of_e]) * 128 + r
    # idx stream (wrap-16, replicated to 8 stripes): global stream col base of
    # call (g,q) = callbase*8
    strm_col = call_col[tile_of_e, q_of_e] * 8 + slot_in_call // 16
    strm_part = slot_in_call % 16

    # tile-major gdst position
    tloc = np.concatenate([np.zeros((NT, 1), np.int64),
                           np.cumsum(pc, axis=1)], axis=1)  # [NT, NQ+1]
    gd_col = (toffs[tile_of_e] + tloc[tile_of_e, q_of_e] + r // 128)
    gd_part = r % 128

    S_total = C_total * 8
    gidx16 = np.zeros((N_CORES, 16, S_total), dtype=np.int16)
    gdst = np.full((N_CORES, 128, C_total), -1.0, dtype=np.float32)
    src_local = (e_src % QR).astype(np.int16)
    dst_local128 = (e_dst % NPC) % 128
    for c in range(N_CORES):
        m = core_of_e == c
        gidx16[c, strm_part[m], strm_col[m]] = src_local[m]
        gdst[c, gd_part[m], gd_col[m]] = dst_local128[m].astype(np.float32)
    gidx16 = np.tile(gidx16, (1, 8, 1))                # replicate stripes

    import ml_dtypes
    gdst = gdst.astype(ml_dtypes.bfloat16)

    # per-core local batch ids, [128, NT], padding rows = -1
    blocal = np.full((N_CORES, 128, NT), -1.0, dtype=np.float32)
    b = np.asarray(batch).astype(np.int64)
    for c in range(N_CORES):
        ids = b[c * NPC:(c + 1) * NPC].astype(np.float32)
        ids = np.concatenate([ids, np.full(NT * 128 - NPC, -1.0, np.float32)])
        blocal[c] = ids.reshape(NT, 128).T
    blocal = blocal.astype(ml_dtypes.bfloat16)

    cnt = np.bincount(b, minlength=NUM_GRAPHS).astype(np.float64)
    inv = (1.0 / np.maximum(cnt, 1.0)).astype(np.float32)

    layout = {
        "pc": pc, "n_t": n_t, "groups": groups, "toffs": toffs,
        "C_total": C_total, "S_total": S_total,
        "gb_groupbase": gb_groupbase, "gb_callbase": gb_callbase,
        "gb_tilebase": gb_tilebase,
    }
    return gidx16, gdst, layout, blocal, inv


def _build(layout):
    from concourse import bacc, bass, mybir
    import concourse.tile as tile

    f32 = mybir.dt.float32
    bf16 = mybir.dt.bfloat16
    i16 = mybir.dt.int16

    pc = layout["pc"]
    groups = layout["groups"]
    toffs = layout["toffs"]
    C_total = layout["C_total"]
    S_total = layout["S_total"]
    gb_groupbase = layout["gb_groupbase"]
    gb_callbase = layout["gb_callbase"]
    gb_tilebase = layout["gb_tilebase"]

    nc = bacc.Bacc("TRN2", target_bir_lowering=False, debug=False,
                   num_devices=N_CORES)

    # ---- I/O ----
    xbf_in = nc.dram_tensor("x_bf", [N_NODES, F], bf16, kind="ExternalInput")
    xT_in = nc.dram_tensor("xT_own", [F, NPC], f32, kind="ExternalInput")
    gidx_in = nc.dram_tensor("gidx16", [128, S_total], i16,
                             kind="ExternalInput")
    gdst_in = nc.dram_tensor("gdst", [128, C_total], bf16,
                             kind="ExternalInput")
    bl_in = nc.dram_tensor("blocal", [128, NT], bf16, kind="ExternalInput")
    iota_in = nc.dram_tensor("iotab", [128, 128], bf16, kind="ExternalInput")
    ident_in = nc.dram_tensor("identb", [128, 128], bf16, kind="ExternalInput")
    w1_in = nc.dram_tensor("w1cat", [F, NUM_LAYERS * F], bf16,
                           kind="ExternalInput")
    w2_in = nc.dram_tensor("w2cat", [F, NUM_LAYERS * F], bf16,
                           kind="ExternalInput")
    b1_in = nc.dram_tensor("b1t", [F, NUM_LAYERS], f32, kind="ExternalInput")
    b2_in = nc.dram_tensor("b2t", [F, NUM_LAYERS], f32, kind="ExternalInput")
    eps_in = nc.dram_tensor("epsb", [F, NUM_LAYERS], f32, kind="ExternalInput")
    wc1_in = nc.dram_tensor("wc1", [F, F], f32, kind="ExternalInput")
    bc1_in = nc.dram_tensor("bc1c", [F, 1], f32, kind="ExternalInput")
    wc2_in = nc.dram_tensor("wc2", [F, NUM_CLASSES], f32, kind="ExternalInput")
    bc2_in = nc.dram_tensor("bc2c", [NUM_CLASSES, 1], f32,
                            kind="ExternalInput")
    inv_in = nc.dram_tensor("invc", [128, NUM_GRAPHS], f32,
                            kind="ExternalInput")
    out_t = nc.dram_tensor("logits_t", [NUM_CLASSES, NUM_GRAPHS], f32,
                           kind="ExternalOutput")

    # ---- internal DRAM ----
    newx = nc.dram_tensor("newx", [NPC, F], bf16, kind="Internal")
    reps = [nc.dram_tensor(f"x_rep{k}", [N_NODES, F], bf16, kind="Internal")
            for k in range(2)]
    pr_in = nc.dram_tensor("pr_in", [128, NUM_GRAPHS], f32, kind="Internal")
    pr_out = nc.dram_tensor("pr_out", [128, NUM_GRAPHS], f32, kind="Internal")

    rg = [list(range(N_CORES))]

    with tile.TileContext(nc) as tc:
        from contextlib import ExitStack
        ctx = ExitStack()
        const = ctx.enter_context(tc.tile_pool(name="const", bufs=1))
        ipool = ctx.enter_context(tc.tile_pool(name="idx", bufs=2))
        gpool = ctx.enter_context(tc.tile_pool(name="gather", bufs=2))
        opool = ctx.enter_context(tc.tile_pool(name="onehot", bufs=3))
        wpool = ctx.enter_context(tc.tile_pool(name="work", bufs=3))
        psum = ctx.enter_context(tc.tile_pool(name="psum", bufs=2,
                                              space="PSUM"))

        xT_res = const.tile([F, NPC], f32)
        bl_t = const.tile([128, NT], bf16)
        iota_t = const.tile([128, 128], bf16)
        ident_t = const.tile([128, 128], bf16)
        w1_t = const.tile([F, NUM_LAYERS * F], bf16)
        w2_t = const.tile([F, NUM_LAYERS * F], bf16)
        b1_t = const.tile([F, NUM_LAYERS], f32)
        b2_t = const.tile([F, NUM_LAYERS], f32)
        eps_t = const.tile([F, NUM_LAYERS], f32)
        wc1_t = const.tile([F, F], f32)
        bc1_t = const.tile([F, 1], f32)
        wc2_t = const.tile([F, NUM_CLASSES], f32)
        bc2_t = const.tile([NUM_CLASSES, 1], f32)
        inv_t = const.tile([128, NUM_GRAPHS], f32)

        for tle, src_t in [(xT_res, xT_in), (bl_t, bl_in), (iota_t, iota_in),
                           (ident_t, ident_in), (w1_t, w1_in), (w2_t, w2_in),
                           (b1_t, b1_in), (b2_t, b2_in), (eps_t, eps_in),
                           (wc1_t, wc1_in), (bc1_t, bc1_in), (wc2_t, wc2_in),
                           (bc2_t, bc2_in), (inv_t, inv_in)]:
            nc.sync.dma_start(tle[:], src_t[:])

        pool_ps = psum.tile([F, NUM_GRAPHS], f32, tag="pool", bufs=1)

        for layer in range(NUM_LAYERS):
            if layer == 0:
                src_d = xbf_in
            else:
                src_d = reps[(layer - 1) % 2]
            last = layer == NUM_LAYERS - 1

            for g, tl in enumerate(groups):
                g0 = gb_groupbase[g]
                g1 = (gb_groupbase[g + 1] if g + 1 < len(groups)
                      else C_total)
                Cg = g1 - g0

                idxt = ipool.tile([128, Cg * 8], i16, tag="idx")
                nc.sync.dma_start(idxt[:], gidx_in[:, g0 * 8:g1 * 8])
                gdt = ipool.tile([128, Cg], bf16, tag="gdt")
                nc.sync.dma_start(gdt[:], gdst_in[:, g0:g1])

                gbuf = gpool.tile([128, Cg, F], bf16, tag="gbuf")
                for q in range(NQ):
                    a = gb_callbase[g][q] - g0
                    b = (gb_callbase[g][q + 1] - g0 if q + 1 < NQ else Cg)
                    if b <= a:
                        continue
                    nidx = (b - a) * 128
                    nc.gpsimd.dma_gather(
                        out_ap=gbuf[:, a:b, :],
                        in_ap=src_d[q * QR:(q + 1) * QR, :],
                        idxs_ap=idxt[:, a * 8:b * 8],
                        num_idxs=nidx,
                        num_idxs_reg=nidx,
                        elem_size=F,
                        single_packet=False,
                    )

                for t in tl:
                    tw = 128 if t < NT - 1 else LAST_W
                    ts = t * 128
                    nt_ = int(toffs[t + 1] - toffs[t])
                    tb = int(toffs[t]) - g0          # tile-major col (group)

                    oh = opool.tile([128, nt_, 128], bf16, tag="oh")
                    nc.vector.tensor_tensor(
                        out=oh[:],
                        in0=gdt[:, tb:tb + nt_, None].to_broadcast(
                            [128, nt_, 128]),
                        in1=iota_t[:, None, :].to_broadcast([128, nt_, 128]),
                        op=mybir.AluOpType.is_equal,
                    )

                    aggr = psum.tile([F, 128], f32, tag="aggr", bufs=2)
                    k = 0
                    for q in range(NQ):
                        base = gb_tilebase[g][q][t] - g0
                        for j in range(int(pc[t, q])):
                            nc.tensor.matmul(aggr[:], gbuf[:, base + j, :],
                                             oh[:, k, :],
                                             start=(k == 0),
                                             stop=(k == nt_ - 1))
                            k += 1

                    xT_sl = xT_res[:, ts:ts + tw]
                    h = wpool.tile([F, 128], f32, tag="h")
                    nc.vector.tensor_scalar(
                        out=h[:, :tw], in0=xT_sl,
                        scalar1=eps_t[:, layer:layer + 1],
                        scalar2=None, op0=mybir.AluOpType.mult)
                    nc.vector.tensor_tensor(
                        out=h[:, :tw], in0=h[:, :tw], in1=aggr[:, :tw],
                        op=mybir.AluOpType.add)
                    hb = wpool.tile([F, 128], bf16, tag="hb")
                    nc.scalar.activation(hb[:, :tw], h[:, :tw],
                                         mybir.ActivationFunctionType.Copy)

                    p1 = psum.tile([F, 128], f32, tag="p1", bufs=1)
                    nc.tensor.matmul(p1[:, :tw],
                                     w1_t[:, layer * F:(layer + 1) * F],
                                     hb[:, :tw], start=True, stop=True)
                    r1 = wpool.tile([F, 128], bf16, tag="r1")
                    nc.scalar.activation(r1[:, :tw], p1[:, :tw],
                                         mybir.ActivationFunctionType.Relu,
                                         bias=b1_t[:, layer:layer + 1])

                    p2 = psum.tile([F, 128], f32, tag="p2", bufs=1)
                    nc.tensor.matmul(p2[:, :tw],
                                     w2_t[:, layer * F:(layer + 1) * F],
                                     r1[:, :tw], start=True, stop=True)

                    o = wpool.tile([F, 128], f32, tag="o")
                    if layer > 0:
                        nc.vector.tensor_tensor(out=o[:, :tw], in0=p2[:, :tw],
                                                in1=h[:, :tw],
                                                op=mybir.AluOpType.add)
                        nc.scalar.activation(o[:, :tw], o[:, :tw],
                                             mybir.ActivationFunctionType.Relu,
                                             bias=b2_t[:, layer:layer + 1])
                    else:
                        nc.scalar.activation(o[:, :tw], p2[:, :tw],
                                             mybir.ActivationFunctionType.Relu,
                                             bias=b2_t[:, layer:layer + 1])
                    nc.vector.tensor_tensor(out=xT_sl, in0=o[:, :tw],
                                            in1=xT_sl,
                                            op=mybir.AluOpType.add)

                    xb = wpool.tile([F, 128], bf16, tag="xb")
                    nc.scalar.activation(xb[:, :tw], xT_sl,
                                         mybir.ActivationFunctionType.Copy)
                    pt = psum.tile([128, F], bf16, tag="pt", bufs=2)
                    nc.tensor.transpose(out=pt[:tw, :], in_=xb[:, :tw],
                                        identity=ident_t[:])
                    st = wpool.tile([128, F], bf16, tag="st")
                    nc.vector.tensor_copy(st[:tw, :], pt[:tw, :])
                    if not last:
                        nc.sync.dma_start(newx[ts:ts + tw, :], st[:tw, :])
                    else:
                        sel = wpool.tile([128, NUM_GRAPHS], bf16, tag="sel")
                        nc.vector.tensor_tensor(
                            out=sel[:],
                            in0=bl_t[:, t:t + 1].to_broadcast(
                                [128, NUM_GRAPHS]),
                            in1=iota_t[:, :NUM_GRAPHS],
                            op=mybir.AluOpType.is_equal,
                        )
                        nc.tensor.matmul(pool_ps[:], st[:], sel[:],
                                         start=(t == 0), stop=(t == NT - 1))

            if not last:
                nc.gpsimd.collective_compute(
                    "AllGather", mybir.AluOpType.bypass,
                    ins=[newx[:]], outs=[reps[layer % 2][:]],
                    replica_groups=rg)

        # ---- mean pool + classifier ----
        pacc = wpool.tile([128, NUM_GRAPHS], f32, tag="pacc")
        nc.vector.tensor_tensor(out=pacc[:], in0=pool_ps[:], in1=inv_t[:],
                                op=mybir.AluOpType.mult)
        nc.sync.dma_start(pr_in[:], pacc[:])
        nc.gpsimd.collective_compute(
            "AllReduce", mybir.AluOpType.add,
            ins=[pr_in[:]], outs=[pr_out[:]], replica_groups=rg)
        pooled = wpool.tile([128, NUM_GRAPHS], f32, tag="pooled")
        nc.sync.dma_start(pooled[:], pr_out[:])

        pc1 = psum.tile([F, NUM_GRAPHS], f32, tag="aggr", bufs=2)
        nc.tensor.matmul(pc1[:], wc1_t[:], pooled[:], start=True, stop=True)
        rc1 = wpool.tile([F, NUM_GRAPHS], f32, tag="rc1")
        nc.scalar.activation(rc1[:], pc1[:],
                             mybir.ActivationFunctionType.Relu,
                             bias=bc1_t[:])
        pc2 = psum.tile([NUM_CLASSES, NUM_GRAPHS], f32, tag="p1", bufs=1)
        nc.tensor.matmul(pc2[:], wc2_t[:], rc1[:], start=True, stop=True)
        lg = wpool.tile([NUM_CLASSES, NUM_GRAPHS], f32, tag="lg")
        nc.vector.tensor_scalar(out=lg[:], in0=pc2[:], scalar1=bc2_t[:],
                                scalar2=None, op0=mybir.AluOpType.add)
        nc.sync.dma_start(out_t[:], lg[:])
        ctx.close()

    nc.compile()
    return nc


def _get_module(layout):
    key = (tuple(layout["pc"].ravel().tolist()), NUM_LAYERS)
    if key not in _CACHE:
        _CACHE.clear()
        _CACHE[key] = _build(layout)
    return _CACHE[key]


def kernel(x, edge_index, batch, eps, W1, b1, W2, b2, Wc1, bc1, Wc2, bc2,
           _trace=False):
    from concourse.bass_utils import run_bass_kernel_spmd
    import ml_dtypes

    x = np.ascontiguousarray(np.asarray(x), dtype=np.float32)
    eps = np.asarray(eps, dtype=np.float32)
    W1 = np.asarray(W1, dtype=np.float32)
    b1 = np.asarray(b1, dtype=np.float32)
    W2 = np.asarray(W2, dtype=np.float32)
    b2 = np.asarray(b2, dtype=np.float32)

    gidx16, gdst, layout, blocal, inv = _preprocess(
        np.asarray(edge_index), np.asarray(batch))
    nc = _get_module(layout)

    L = NUM_LAYERS
    x_bf = np.ascontiguousarray(x.astype(ml_dtypes.bfloat16))
    iota_b = np.ascontiguousarray(
        np.broadcast_to(np.arange(128, dtype=np.float32),
                        (128, 128))).astype(ml_dtypes.bfloat16)
    common = {
        "x_bf": x_bf,
        "iotab": iota_b,
        "identb": np.eye(128, dtype=np.float32).astype(ml_dtypes.bfloat16),
        "w1cat": np.ascontiguousarray(
            np.concatenate(list(W1[:L]), axis=1)).astype(ml_dtypes.bfloat16),
        "w2cat": np.ascontiguousarray(
            np.concatenate(list(W2[:L]), axis=1)).astype(ml_dtypes.bfloat16),
        "b1t": np.ascontiguousarray(b1[:L].T),
        "b2t": np.ascontiguousarray(b2[:L].T),
        "epsb": np.ascontiguousarray(
            np.broadcast_to(1.0 + eps[:L], (F, L))),
        "wc1": np.ascontiguousarray(np.asarray(Wc1, np.float32)),
        "bc1c": np.ascontiguousarray(np.asarray(bc1, np.float32)[:, None]),
        "wc2": np.ascontiguousarray(np.asarray(Wc2, np.float32)),
        "bc2c": np.ascontiguousarray(np.asarray(bc2, np.float32)[:, None]),
        "invc": np.ascontiguousarray(np.broadcast_to(inv, (128, NUM_GRAPHS))),
    }
    in_maps = []
    for c in range(N_CORES):
        m = dict(common)
        m["xT_own"] = np.ascontiguousarray(x[c * NPC:(c + 1) * NPC].T)
        m["gidx16"] = np.ascontiguousarray(gidx16[c])
        m["gdst"] = np.ascontiguousarray(gdst[c])
        m["blocal"] = np.ascontiguousarray(blocal[c])
        in_maps.append(m)

    res = run_bass_kernel_spmd(nc, in_maps, core_ids=list(range(N_CORES)),
                               trace=_trace)
    out = np.ascontiguousarray(res.results[0]["logits_t"].T)
    if _trace:
        kernel._last_result = res
    return out


# revision 8
# speedup vs baseline: 1.2934x; 1.0080x over previous
"""GIN message-passing GNN on 8 Trainium2 NeuronCores (Bass/Tile).

V2 strategy (self-contained; shapes hardcoded for the 100k-node / 1.6M-edge /
128-dim / 10-layer / 64-graph problem):

- Nodes partitioned into 8 contiguous ranges of 12500; each core owns the
  edges whose destination falls in its range. Node state is kept in bf16.
- Per layer, each core gathers x[src] for its ~200k edges with a few large
  GPSIMD dma_gather calls (int16 indices => sources are split into 4
  "quarters" of 25000 nodes; per destination-tile the edges are bucketed by
  source quarter, each (tile, quarter) bucket padded to a 128-edge-chunk
  boundary with dummy index 0; padding is the max over cores so the program
  is SPMD). Calls cover groups of G tiles -> ~4*ceil(98/G) gather
  instructions per layer instead of ~1600 indirect DMAs.
- The segment-sum becomes a matmul per 128-edge chunk:
  PSUM[feat, dst] += contract_edges(gathered[edge, feat], onehot[edge, dst])
  with the bf16 one-hot built on the vector engine by is_equal against a
  resident iota row (dummy slots have dst=-1 -> zero column).
- The GIN MLP runs in the transposed [feat, node] orientation in bf16 (fp32
  PSUM accumulation); the per-core x^T slab stays resident in SBUF in fp32
  for exact residual adds. Output tiles are cast to bf16, transposed on the
  tensor engine, and stored to a local newx slab.
- After each layer an AllGather with a *Shared* HBM output rebuilds the full
  replica once (each core only contributes its 3.2MB slab instead of
  materializing a private 25.6MB copy). Replicas rotate over 3 shared
  buffers to avoid cross-layer write-after-read hazards.
- Mean-pool: last layer's output tiles are reduced into PSUM[feat, graph]
  via a matmul against a graph one-hot; scale by 1/count, AllReduce, then
  the small classifier MLP on every core.
"""
import os
import sys

sys.path.insert(0, "/opt/trn_rl_repo")

import numpy as np

N_NODES = 100000
N_EDGES = 1600000
F = 128
NUM_LAYERS = int(os.environ.get("GNN_LAYERS", "10"))
NUM_GRAPHS = 64
NUM_CLASSES = 2
N_CORES = 8
NPC = N_NODES // N_CORES          # 12500 nodes per core
NT = (NPC + 127) // 128           # 98 dst tiles per core
LAST_W = NPC - (NT - 1) * 128     # 84 nodes in the last tile
NQ = 4                            # source quarters (int16 idx limit)
QR = N_NODES // NQ                # 25000 rows per quarter
GTILES = 6                        # dst tiles per gather group

_CACHE = {}


def _preprocess(edge_index, batch):
    """Host-side: per-core gather index streams / dst-offset streams.

    Returns (gidx16, gdst, layout, blocal, inv) where layout carries all the
    static column bookkeeping shared by every core.
    """
    src = np.ascontiguousarray(edge_index[0]).astype(np.int64)
    dst = np.ascontiguousarray(edge_index[1]).astype(np.int64)

    # sort edges by (core, tile, quarter) with one composite stable sort
    gt = (dst // NPC) * NT + (dst % NPC) // 128      # global tile 0..783
    key = gt * NQ + src // QR
    order = np.argsort(key, kind="stable")
    e_src = src[order]
    e_dst = dst[order]
    e_key = key[order]

    counts = np.bincount(e_key, minlength=N_CORES * NT * NQ)
    counts = counts.reshape(N_CORES, NT, NQ)
    pc = (counts.max(axis=0) + 127) // 128            # [NT, NQ] chunks
    n_t = pc.sum(axis=1)                              # [NT]

    # group layout: group g covers tiles [g*GTILES, ...); within a group the
    # gather-call (gbuf) column order is q-major: q0:[t0,t1,..] q1:[...] ...
    ngroups = (NT + GTILES - 1) // GTILES
    groups = [list(range(g * GTILES, min((g + 1) * GTILES, NT)))
              for g in range(ngroups)]

    # tile-major (one-hot / gdst) global column base
    toffs = np.concatenate([[0], np.cumsum(n_t)]).astype(np.int64)
    C_total = int(toffs[-1])

    # gbuf column bookkeeping
    gb_groupbase = []          # global gbuf col where group g starts
    gb_callbase = []           # [g][q] global gbuf col where call (g,q) starts
    gb_tilebase = []           # [g][q][t] global gbuf col of tile t's q-chunks
    pos = 0
    for g, tl in enumerate(groups):
        gb_groupbase.append(pos)
        cb = []
        tb = []
        for q in range(NQ):
            cb.append(pos)
            tbq = {}
            for t in tl:
                tbq[t] = pos
                pos += int(pc[t, q])
            tb.append(tbq)
        gb_callbase.append(cb)
        gb_tilebase.append(tb)
    assert pos == C_total

    # per-edge positions (same math for every core; data differs)
    g_of_t = np.zeros(NT, dtype=np.int64)
    for g, tl in enumerate(groups):
        for t in tl:
            g_of_t[t] = g
    tile_of_e = (e_dst % NPC) // 128
    q_of_e = e_src // QR
    core_of_e = e_dst // NPC

    # rank of each edge within its (c,t,q) bucket
    bucket_sizes = counts.reshape(-1)
    bucket_start = np.concatenate([[0], np.cumsum(bucket_sizes)])[:-1]
    flat_key = (core_of_e * NT + tile_of_e) * NQ + q_of_e
    r = np.arange(len(e_src)) - bucket_start[flat_key]

    # gbuf slot (within-call): (tile's col base - call base)*128 + r
    gb_tile_col = np.zeros((NT, NQ), dtype=np.int64)
    call_col = np.zeros((NT, NQ), dtype=np.int64)
    for g, tl in enumerate(groups):
        for q in range(NQ):
            for t in tl:
                gb_tile_col[t, q] = gb_tilebase[g][q][t]
                call_col[t, q] = gb_callbase[g][q]
    slot_in_call = (gb_tile_col[tile_of_e, q_of_e]
                    - call_col[tile_of_e, q_of_e]) * 128 + r
    # idx stream (wrap-16, replicated to 8 stripes): global stream col base of
    # call (g,q) = callbase*8
    strm_col = call_col[tile_of_e, q_of_e] * 8 + slot_in_call // 16
    strm_part = slot_in_call % 16

    # tile-major gdst position
    tloc = np.concatenate([np.zeros((NT, 1), np.int64),
                           np.cumsum(pc, axis=1)], axis=1)  # [NT, NQ+1]
    gd_col = (toffs[tile_of_e] + tloc[tile_of_e, q_of_e] + r // 128)
    gd_part = r % 128

    S_total = C_total * 8
    gidx16 = np.zeros((N_CORES, 16, S_total), dtype=np.int16)
    gdst = np.full((N_CORES, 128, C_total), -1.0, dtype=np.float32)
    src_local = (e_src % QR).astype(np.int16)
    dst_local128 = (e_dst % NPC) % 128
    for c in range(N_CORES):
        m = core_of_e == c
        gidx16[c, strm_part[m], strm_col[m]] = src_local[m]
        gdst[c, gd_part[m], gd_col[m]] = dst_local128[m].astype(np.float32)
    gidx16 = np.tile(gidx16, (1, 8, 1))                # replicate stripes

    import ml_dtypes
    gdst = gdst.astype(ml_dtypes.bfloat16)

    # per-core local batch ids, [128, NT], padding rows = -1
    blocal = np.full((N_CORES, 128, NT), -1.0, dtype=np.float32)
    b = np.asarray(batch).astype(np.int64)
    for c in range(N_CORES):
        ids = b[c * NPC:(c + 1) * NPC].astype(np.float32)
        ids = np.concatenate([ids, np.full(NT * 128 - NPC, -1.0, np.float32)])
        blocal[c] = ids.reshape(NT, 128).T
    blocal = blocal.astype(ml_dtypes.bfloat16)

    cnt = np.bincount(b, minlength=NUM_GRAPHS).astype(np.float64)
    inv = (1.0 / np.maximum(cnt, 1.0)).astype(np.float32)

    layout = {
        "pc": pc, "n_t": n_t, "groups": groups, "toffs": toffs,
        "C_total": C_total, "S_total": S_total,
        "gb_groupbase": gb_groupbase, "gb_callbase": gb_callbase,
        "gb_tilebase": gb_tilebase,
    }
    return gidx16, gdst, layout, blocal, inv


def _build(layout):
    from concourse import bacc, bass, mybir
    import concourse.tile as tile

    f32 = mybir.dt.float32
    bf16 = mybir.dt.bfloat16
    i16 = mybir.dt.int16

    pc = layout["pc"]
    groups = layout["groups"]
    toffs = layout["toffs"]
    C_total = layout["C_total"]
    S_total = layout["S_total"]
    gb_groupbase = layout["gb_groupbase"]
    gb_callbase = layout["gb_callbase"]
    gb_tilebase = layout["gb_tilebase"]

    nc = bacc.Bacc("TRN2", target_bir_lowering=False, debug=False,
                   num_devices=N_CORES)

    # ---- I/O ----
    xbf_in = nc.dram_tensor("x_bf", [N_NODES, F], bf16, kind="ExternalInput")
    xT_in = nc.dram_tensor("xT_own", [F, NPC], f32, kind="ExternalInput")
    gidx_in = nc.dram_tensor("gidx16", [128, S_total], i16,
                             kind="ExternalInput")
    gdst_in = nc.dram_tensor("gdst", [128, C_total], bf16,
                             kind="ExternalInput")
    bl_in = nc.dram_tensor("blocal", [128, NT], bf16, kind="ExternalInput")
    iota_in = nc.dram_tensor("iotab", [128, 128], bf16, kind="ExternalInput")
    ident_in = nc.dram_tensor("identb", [128, 128], bf16, kind="ExternalInput")
    w1_in = nc.dram_tensor("w1cat", [F, NUM_LAYERS * F], bf16,
                           kind="ExternalInput")
    w2_in = nc.dram_tensor("w2cat", [F, NUM_LAYERS * F], bf16,
                           kind="ExternalInput")
    b1_in = nc.dram_tensor("b1t", [F, NUM_LAYERS], f32, kind="ExternalInput")
    b2_in = nc.dram_tensor("b2t", [F, NUM_LAYERS], f32, kind="ExternalInput")
    eps_in = nc.dram_tensor("epsb", [F, NUM_LAYERS], f32, kind="ExternalInput")
    wc1_in = nc.dram_tensor("wc1", [F, F], f32, kind="ExternalInput")
    bc1_in = nc.dram_tensor("bc1c", [F, 1], f32, kind="ExternalInput")
    wc2_in = nc.dram_tensor("wc2", [F, NUM_CLASSES], f32, kind="ExternalInput")
    bc2_in = nc.dram_tensor("bc2c", [NUM_CLASSES, 1], f32,
                            kind="ExternalInput")
    inv_in = nc.dram_tensor("invc", [128, NUM_GRAPHS], f32,
                            kind="ExternalInput")
    out_t = nc.dram_tensor("logits_t", [NUM_CLASSES, NUM_GRAPHS], f32,
                           kind="ExternalOutput")

    # ---- internal DRAM ----
    newx = nc.dram_tensor("newx", [NPC, F], bf16, kind="Internal")
    reps = [nc.dram_tensor(f"x_rep{k}", [N_NODES, F], bf16, kind="Internal")
            for k in range(2)]
    # one-hot scatter matrices, built on-device in layer 0, streamed back in
    # later layers (the graph is static -> they never change)
    f8 = mybir.dt.float8e4
    oh_d = nc.dram_tensor("oh_d", [128, C_total, 128], f8, kind="Internal")
    pr_in = nc.dram_tensor("pr_in", [128, NUM_GRAPHS], f32, kind="Internal")
    pr_out = nc.dram_tensor("pr_out", [128, NUM_GRAPHS], f32, kind="Internal")

    rg = [list(range(N_CORES))]

    with tile.TileContext(nc) as tc:
        from contextlib import ExitStack
        ctx = ExitStack()
        const = ctx.enter_context(tc.tile_pool(name="const", bufs=1))
        ipool = ctx.enter_context(tc.tile_pool(name="idx", bufs=2))
        gpool = ctx.enter_context(tc.tile_pool(name="gather", bufs=2))
        opool = ctx.enter_context(tc.tile_pool(name="onehot", bufs=3))
        wpool = ctx.enter_context(tc.tile_pool(name="work", bufs=3))
        psum = ctx.enter_context(tc.tile_pool(name="psum", bufs=2,
                                              space="PSUM"))

        xT_res = const.tile([F, NPC], f32)
        bl_t = const.tile([128, NT], bf16)
        iota_t = const.tile([128, 128], bf16)
        ident_t = const.tile([128, 128], bf16)
        w1_t = const.tile([F, NUM_LAYERS * F], bf16)
        w2_t = const.tile([F, NUM_LAYERS * F], bf16)
        b1_t = const.tile([F, NUM_LAYERS], f32)
        b2_t = const.tile([F, NUM_LAYERS], f32)
        eps_t = const.tile([F, NUM_LAYERS], f32)
        wc1_t = const.tile([F, F], f32)
        bc1_t = const.tile([F, 1], f32)
        wc2_t = const.tile([F, NUM_CLASSES], f32)
        bc2_t = const.tile([NUM_CLASSES, 1], f32)
        inv_t = const.tile([128, NUM_GRAPHS], f32)

        for tle, src_t in [(xT_res, xT_in), (bl_t, bl_in), (iota_t, iota_in),
                           (ident_t, ident_in), (w1_t, w1_in), (w2_t, w2_in),
                           (b1_t, b1_in), (b2_t, b2_in), (eps_t, eps_in),
                           (wc1_t, wc1_in), (bc1_t, bc1_in), (wc2_t, wc2_in),
                           (bc2_t, bc2_in), (inv_t, inv_in)]:
            nc.sync.dma_start(tle[:], src_t[:])

        pool_ps = psum.tile([F, NUM_GRAPHS], f32, tag="pool", bufs=1)

        for layer in range(NUM_LAYERS):
            if layer == 0:
                src_d = xbf_in
            else:
                src_d = reps[(layer - 1) % 2]
            last = layer == NUM_LAYERS - 1

            for g, tl in enumerate(groups):
                g0 = gb_groupbase[g]
                g1 = (gb_groupbase[g + 1] if g + 1 < len(groups)
                      else C_total)
                Cg = g1 - g0

                idxt = ipool.tile([128, Cg * 8], i16, tag="idx")
                nc.sync.dma_start(idxt[:], gidx_in[:, g0 * 8:g1 * 8])
                if layer == 0:
                    gdt = ipool.tile([128, Cg], bf16, tag="gdt")
                    nc.sync.dma_start(gdt[:], gdst_in[:, g0:g1])

                gbuf = gpool.tile([128, Cg, F], bf16, tag="gbuf")
                for q in range(NQ):
                    a = gb_callbase[g][q] - g0
                    b = (gb_callbase[g][q + 1] - g0 if q + 1 < NQ else Cg)
                    if b <= a:
                        continue
                    nidx = (b - a) * 128
                    nc.gpsimd.dma_gather(
                        out_ap=gbuf[:, a:b, :],
                        in_ap=src_d[q * QR:(q + 1) * QR, :],
                        idxs_ap=idxt[:, a * 8:b * 8],
                        num_idxs=nidx,
                        num_idxs_reg=nidx,
                        elem_size=F,
                        single_packet=False,
                    )

                for t in tl:
                    tw = 128 if t < NT - 1 else LAST_W
                    ts = t * 128
                    nt_ = int(toffs[t + 1] - toffs[t])
                    tb = int(toffs[t]) - g0          # tile-major col (group)

                    oh = opool.tile([128, nt_, 128], f8, tag="oh")
                    if layer == 0:
                        nc.vector.tensor_tensor(
                            out=oh[:],
                            in0=gdt[:, tb:tb + nt_, None].to_broadcast(
                                [128, nt_, 128]),
                            in1=iota_t[:, None, :].to_broadcast(
                                [128, nt_, 128]),
                            op=mybir.AluOpType.is_equal,
                        )
                        nc.sync.dma_start(
                            oh_d[:, g0 + tb:g0 + tb + nt_, :], oh[:])
                    else:
                        nc.sync.dma_start(
                            oh[:], oh_d[:, g0 + tb:g0 + tb + nt_, :])

                    aggr = psum.tile([F, 128], f32, tag="aggr", bufs=2)
                    k = 0
                    for q in range(NQ):
                        base = gb_tilebase[g][q][t] - g0
                        for j in range(int(pc[t, q])):
                            nc.tensor.matmul(aggr[:], gbuf[:, base + j, :],
                                             oh[:, k, :],
                                             start=(k == 0),
                                             stop=(k == nt_ - 1))
                            k += 1

                    xT_sl = xT_res[:, ts:ts + tw]
                    h = wpool.tile([F, 128], f32, tag="h")
                    nc.vector.tensor_scalar(
                        out=h[:, :tw], in0=xT_sl,
                        scalar1=eps_t[:, layer:layer + 1],
                        scalar2=None, op0=mybir.AluOpType.mult)
                    nc.vector.tensor_tensor(
                        out=h[:, :tw], in0=h[:, :tw], in1=aggr[:, :tw],
                        op=mybir.AluOpType.add)
                    hb = wpool.tile([F, 128], bf16, tag="hb")
                    nc.scalar.activation(hb[:, :tw], h[:, :tw],
                                         mybir.ActivationFunctionType.Copy)

                    p1 = psum.tile([F, 128], f32, tag="p1", bufs=1)
                    nc.tensor.matmul(p1[:, :tw],
                                     w1_t[:, layer * F:(layer + 1) * F],
                                     hb[:, :tw], start=True, stop=True)
                    r1 = wpool.tile([F, 128], bf16, tag="r1")
                    nc.scalar.activation(r1[:, :tw], p1[:, :tw],
                                         mybir.ActivationFunctionType.Relu,
                                         bias=b1_t[:, layer:layer + 1])

                    p2 = psum.tile([F, 128], f32, tag="p2", bufs=1)
                    nc.tensor.matmul(p2[:, :tw],
                                     w2_t[:, layer * F:(layer + 1) * F],
                                     r1[:, :tw], start=True, stop=True)

                    o = wpool.tile([F, 128], f32, tag="o")
                    if layer > 0:
                        nc.vector.tensor_tensor(out=o[:, :tw], in0=p2[:, :tw],
                                                in1=h[:, :tw],
                                                op=mybir.AluOpType.add)
                        nc.scalar.activation(o[:, :tw], o[:, :tw],
                                             mybir.ActivationFunctionType.Relu,
                                             bias=b2_t[:, layer:layer + 1])
                    else:
                        nc.scalar.activation(o[:, :tw], p2[:, :tw],
                                             mybir.ActivationFunctionType.Relu,
                                             bias=b2_t[:, layer:layer + 1])
                    nc.vector.tensor_tensor(out=xT_sl, in0=o[:, :tw],
                                            in1=xT_sl,
                                            op=mybir.AluOpType.add)

                    xb = wpool.tile([F, 128], bf16, tag="xb")
                    nc.scalar.activation(xb[:, :tw], xT_sl,
                                         mybir.ActivationFunctionType.Copy)
                    pt = psum.tile([128, F], bf16, tag="pt", bufs=2)
                    nc.tensor.transpose(out=pt[:tw, :], in_=xb[:, :tw],
                                        identity=ident_t[:])
                    st = wpool.tile([128, F], bf16, tag="st")
                    nc.vector.tensor_copy(st[:tw, :], pt[:tw, :])
                    if not last:
                        nc.sync.dma_start(newx[ts:ts + tw, :], st[:tw, :])
                    else:
                        sel = wpool.tile([128, NUM_GRAPHS], bf16, tag="sel")
                        nc.vector.tensor_tensor(
                            out=sel[:],
                            in0=bl_t[:, t:t + 1].to_broadcast(
                                [128, NUM_GRAPHS]),
                            in1=iota_t[:, :NUM_GRAPHS],
                            op=mybir.AluOpType.is_equal,
                        )
                        nc.tensor.matmul(pool_ps[:], st[:], sel[:],
                                         start=(t == 0), stop=(t == NT - 1))

            if not last:
                nc.gpsimd.collective_compute(
                    "AllGather", mybir.AluOpType.bypass,
                    ins=[newx[:]], outs=[reps[layer % 2][:]],
                    replica_groups=rg)

        # ---- mean pool + classifier ----
        pacc = wpool.tile([128, NUM_GRAPHS], f32, tag="pacc")
        nc.vector.tensor_tensor(out=pacc[:], in0=pool_ps[:], in1=inv_t[:],
                                op=mybir.AluOpType.mult)
        nc.sync.dma_start(pr_in[:], pacc[:])
        nc.gpsimd.collective_compute(
            "AllReduce", mybir.AluOpType.add,
            ins=[pr_in[:]], outs=[pr_out[:]], replica_groups=rg)
        pooled = wpool.tile([128, NUM_GRAPHS], f32, tag="pooled")
        nc.sync.dma_start(pooled[:], pr_out[:])

        pc1 = psum.tile([F, NUM_GRAPHS], f32, tag="aggr", bufs=2)
        nc.tensor.matmul(pc1[:], wc1_t[:], pooled[:], start=True, stop=True)
        rc1 = wpool.tile([F, NUM_GRAPHS], f32, tag="rc1")
        nc.scalar.activation(rc1[:], pc1[:],
                             mybir.ActivationFunctionType.Relu,
                             bias=bc1_t[:])
        pc2 = psum.tile([NUM_CLASSES, NUM_GRAPHS], f32, tag="p1", bufs=1)
        nc.tensor.matmul(pc2[:], wc2_t[:], rc1[:], start=True, stop=True)
        lg = wpool.tile([NUM_CLASSES, NUM_GRAPHS], f32, tag="lg")
        nc.vector.tensor_scalar(out=lg[:], in0=pc2[:], scalar1=bc2_t[:],
                                scalar2=None, op0=mybir.AluOpType.add)
        nc.sync.dma_start(out_t[:], lg[:])
        ctx.close()

    nc.compile()
    return nc


def _get_module(layout):
    key = (tuple(layout["pc"].ravel().tolist()), NUM_LAYERS)
    if key not in _CACHE:
        _CACHE.clear()
        _CACHE[key] = _build(layout)
    return _CACHE[key]


def kernel(x, edge_index, batch, eps, W1, b1, W2, b2, Wc1, bc1, Wc2, bc2,
           _trace=False):
    from concourse.bass_utils import run_bass_kernel_spmd
    import ml_dtypes

    x = np.ascontiguousarray(np.asarray(x), dtype=np.float32)
    eps = np.asarray(eps, dtype=np.float32)
    W1 = np.asarray(W1, dtype=np.float32)
    b1 = np.asarray(b1, dtype=np.float32)
    W2 = np.asarray(W2, dtype=np.float32)
    b2 = np.asarray(b2, dtype=np.float32)

    gidx16, gdst, layout, blocal, inv = _preprocess(
        np.asarray(edge_index), np.asarray(batch))
    nc = _get_module(layout)

    L = NUM_LAYERS
    x_bf = np.ascontiguousarray(x.astype(ml_dtypes.bfloat16))
    iota_b = np.ascontiguousarray(
        np.broadcast_to(np.arange(128, dtype=np.float32),
                        (128, 128))).astype(ml_dtypes.bfloat16)
    common = {
        "x_bf": x_bf,
        "iotab": iota_b,
        "identb": np.eye(128, dtype=np.float32).astype(ml_dtypes.bfloat16),
        "w1cat": np.ascontiguousarray(
            np.concatenate(list(W1[:L]), axis=1)).astype(ml_dtypes.bfloat16),
        "w2cat": np.ascontiguousarray(
            np.concatenate(list(W2[:L]), axis=1)).astype(ml_dtypes.bfloat16),
        "b1t": np.ascontiguousarray(b1[:L].T),
        "b2t": np.ascontiguousarray(b2[:L].T),
        "epsb": np.ascontiguousarray(
            np.broadcast_to(1.0 + eps[:L], (F, L))),
        "wc1": np.ascontiguousarray(np.asarray(Wc1, np.float32)),
        "bc1c": np.ascontiguousarray(np.asarray(bc1, np.float32)[:, None]),
        "wc2": np.ascontiguousarray(np.asarray(Wc2, np.float32)),
        "bc2c": np.ascontiguousarray(np.asarray(bc2, np.float32)[:, None]),
        "invc": np.ascontiguousarray(np.broadcast_to(inv, (128, NUM_GRAPHS))),
    }
    in_maps = []
    for c in range(N_CORES):
        m = dict(common)
        m["xT_own"] = np.ascontiguousarray(x[c * NPC:(c + 1) * NPC].T)
        m["gidx16"] = np.ascontiguousarray(gidx16[c])
        m["gdst"] = np.ascontiguousarray(gdst[c])
        m["blocal"] = np.ascontiguousarray(blocal[c])
        in_maps.append(m)

    res = run_bass_kernel_spmd(nc, in_maps, core_ids=list(range(N_CORES)),
                               trace=_trace)
    out = np.ascontiguousarray(res.results[0]["logits_t"].T)
    if _trace:
        kernel._last_result = res
    return out
